# revision 1
# baseline (speedup 1.0000x reference)
"""Trainium2 Bass kernel for the AttentionLayer problem.

Computation (per batch b):
    q = query[b] @ Wq + bq            [S, A]
    v = value[b] @ Wv + bv            [S, A]
    scores = q @ v.T                  [S, S]
    attn = softmax(scores, -1)
    out[b] = attn @ v                 [S, A]

with B=4, S=2048, HIDDEN=A=1024, fp32 reference; B*S*S*A dominates.

Sharding: 8 cores = (batch b in 0..3) x (query-row half h in 0..1).
Each core handles 1024 query rows of one batch and computes the full
v projection for its batch (duplicated across the pair of cores
sharing a batch; avoids collectives).

Key design points vs the previous f32r version (242 us measured):
  - All matmul operands are fp16 (same 1 cycle/row PE throughput as
    f32r, but fp32-accumulated in PSUM). Host converts inputs to fp16,
    halving HBM traffic.
  - ZERO PE transposes: query/value are loaded pre-transposed via the
    DMA XBAR (dma_start_transpose, 2-byte dtypes), v (s-major) is
    produced from vT by an SBUF->SBUF DMA transpose, and attn^T for
    the context matmul by a per-q-tile SBUF->SBUF DMA transpose.
    The old kernel spent ~107k PE cycles (~45 us) on identity-matmul
    transposes; these now ride the otherwise idle DMA engines.
  - Softmax is row-max-stabilized (DVE negated max reduce feeds the
    ACT Exp bias) so exp() fits fp16; 1/rowsum is folded into the
    context PSUM->SBUF copy-out (DVE per-partition tensor_scalar).
  - Attention is software-pipelined: PE order sc(0), sc(1), cx(0),
    sc(2), cx(1), ... so the max/exp/transpose latency of tile i hides
    under the score matmul of tile i+1. PSUM: sc [128,2048] bufs=1
    (4 banks) + cx [128,1024] bufs=2 (4 banks).

PE work per core: qproj 65536 + vproj 131072 + scores 131072 +
context 131072 = 458752 cycles ~= 191 us at 2.4 GHz peak.
"""

import sys

if "/opt/trn_rl_repo" not in sys.path:
    sys.path.insert(0, "/opt/trn_rl_repo")

import numpy as np

import concourse.bass as bass
import concourse.mybir as mybir
from concourse import bacc, tile
from concourse.bass_utils import run_bass_kernel_spmd

F32 = mybir.dt.float32
F16 = mybir.dt.float16

B, S, H, A = 4, 2048, 1024, 1024
SQ = S // 2  # query rows per core
P = 128
N_CORES = 8
KO = H // P  # 8 contraction chunks of 128
AO = A // P  # 8 a-tiles
SO = S // P  # 16 key tiles
QO = SQ // P  # 8 query tiles per core

Exp = mybir.ActivationFunctionType.Exp
Identity = mybir.ActivationFunctionType.Identity
AxX = mybir.AxisListType.X
MaxOp = mybir.AluOpType.max


TP = True  # tensor-parallel v-projection (pairwise AllGather of vT halves)
AH = AO // 2  # a-tiles computed locally per core when TP


def build(repeat: int = 1, rp: int = 1, ra: int = 1, tp: bool | None = None):
    """repeat: whole-kernel repetitions (timing). rp/ra: projection-phase /
    attention-phase inner repetitions (phase-isolation diagnostics)."""
    if tp is None:
        tp = TP
    nc = bacc.Bacc(None, target_bir_lowering=False, debug=False)

    a_loc = A // 2 if tp else A  # local Wv columns
    ao_loc = AH if tp else AO

    xq = nc.dram_tensor("xq", [SQ, H], F16, kind="ExternalInput")
    xv = nc.dram_tensor("xv", [S, H], F16, kind="ExternalInput")
    wq = nc.dram_tensor("wq", [P, KO, A], F16, kind="ExternalInput")
    wv = nc.dram_tensor("wv", [P, KO, a_loc], F16, kind="ExternalInput")
    bq = nc.dram_tensor("bq", [P, AO], F32, kind="ExternalInput")
    bv = nc.dram_tensor("bv", [P, ao_loc], F32, kind="ExternalInput")
    # fp16 output (host upcasts to fp32; ~2.4e-4 extra rounding, halves the
    # output DMA and doubles the DVE copy-out rate)
    out = nc.dram_tensor("out", [SQ, A], F16, kind="ExternalOutput")
    out_t = out.rearrange("(o p) f -> o p f", p=P)  # [8, 128, 1024]

    with tile.TileContext(nc) as tc:
        with tc.tile_pool(name="pers", bufs=1) as pers:
            bq_sb = pers.tile([P, AO], F32, name="bq_sb")
            nc.sync.dma_start(bq_sb[:], bq[:])
            bv_sb = pers.tile([P, ao_loc], F32, name="bv_sb")
            nc.sync.dma_start(bv_sb[:], bv[:])

            # persistent activations (a-major / s-major), fp16
            qT = pers.tile([P, AO, SQ], F16, name="qT", tag="qT")  # 16KB/part
            vT = pers.tile([P, AO, S], F16, name="vT", tag="vT")  # 32KB
            v_sb = pers.tile([P, SO, A], F16, name="v_sb", tag="v")  # 32KB

            for _rep in range(repeat):
              for _rp in range(rp):
                proj = tc.alloc_tile_pool(name="proj", bufs=1)
                psp = tc.alloc_tile_pool(name="psp", bufs=1, space="PSUM")
                if tp:
                    dram = tc.alloc_tile_pool(name="dram", bufs=1, space="DRAM")
                    cc_in = dram.tile([P, AH, S], F16)
                    cc_out = dram.tile([2, P, AH, S], F16)

                # ---- input loads on the ACT hwdge queue (the SP queue is
                # reserved for the chain-critical attn transposes so next-
                # iteration loads can stream during attention without
                # blocking them). v path first when TP: vproj starts the
                # collective as early as possible; q path fills the gap. ----
                wv_sb = proj.tile([P, KO, a_loc], F16, name="wv_sb", tag="wv")
                nc.scalar.dma_start(wv_sb[:], wv[:])
                valueT = proj.tile([P, KO, S], F16, name="valueT", tag="val")
                for c in range(4):
                    nc.scalar.dma_start_transpose(
                        valueT[:, :, c * 512 : (c + 1) * 512],
                        xv[c * 512 : (c + 1) * 512, :],
                    )
                wq_sb = proj.tile([P, KO, A], F16, name="wq_sb", tag="wq")
                nc.scalar.dma_start(wq_sb[:], wq[:])
                queryT = proj.tile([P, KO, SQ], F16, name="queryT", tag="qry")
                for c in range(2):
                    nc.scalar.dma_start_transpose(
                        queryT[:, :, c * 512 : (c + 1) * 512],
                        xq[c * 512 : (c + 1) * 512, :],
                    )

                def vproj_tile(ao):
                    # local vT a-tile ao (written to slot ao; with TP the
                    # core's half always lands in slots 0..AH-1, the gather
                    # readback then places both halves)
                    pp = psp.tile([P, S], F32, name=f"pv_{ao}", tag="pp", bufs=2)
                    # s-chunk-major: chunk c4 only needs valueT cols
                    # [c4*512, (c4+1)*512) -> PE starts after the first
                    # valueT transpose lands instead of all four
                    for c4 in range(4):
                        for k in range(KO):
                            nc.tensor.matmul(
                                pp[:, c4 * 512 : (c4 + 1) * 512],
                                wv_sb[:, k, ao * P : (ao + 1) * P],
                                valueT[:, k, c4 * 512 : (c4 + 1) * 512],
                                start=(k == 0),
                                stop=(k == KO - 1),
                            )
                    nc.scalar.activation(
                        vT[:, ao, :], pp[:], Identity, bias=bv_sb[:, ao : ao + 1]
                    )
                    if tp:
                        nc.gpsimd.dma_start(cc_in[:, ao, :], vT[:, ao, :])
                    else:
                        nc.scalar.dma_start_transpose(
                            v_sb[:, :, ao * P : (ao + 1) * P], vT[:, ao, :]
                        )

                def qproj_tile(ao):
                    pp = psp.tile([P, S], F32, name=f"pq_{ao}", tag="pp", bufs=2)
                    for k in range(KO):
                        for c2 in range(2):
                            nc.tensor.matmul(
                                pp[:, c2 * 512 : (c2 + 1) * 512],
                                wq_sb[:, k, ao * P : (ao + 1) * P],
                                queryT[:, k, c2 * 512 : (c2 + 1) * 512],
                                start=(k == 0),
                                stop=(k == KO - 1),
                            )
                    nc.scalar.activation(
                        qT[:, ao, :], pp[:, :SQ], Identity, bias=bq_sb[:, ao : ao + 1]
                    )

                if tp:
                    # v-projection half -> AllGather (overlapped with qproj)
                    for ao in range(AH):
                        vproj_tile(ao)
                    nc.gpsimd.collective_compute(
                        "AllGather",
                        mybir.AluOpType.bypass,
                        replica_groups=[[2 * i, 2 * i + 1] for i in range(N_CORES // 2)],
                        ins=[cc_in.opt()],
                        outs=[cc_out.opt()],
                    )
                    for ao in range(AO):
                        qproj_tile(ao)
                    # gather readback into vT slots (chunked, pool queue);
                    # v via DMA transpose (ACT hwdge queue) right behind
                    for r in range(2):
                        for pr in range(2):
                            nc.gpsimd.dma_start(
                                vT[:, r * AH + pr * 2 : r * AH + (pr + 1) * 2, :],
                                cc_out[r][:, pr * 2 : (pr + 1) * 2, :],
                            )
                            for j in range(2):
                                ao = r * AH + pr * 2 + j
                                nc.scalar.dma_start_transpose(
                                    v_sb[:, :, ao * P : (ao + 1) * P], vT[:, ao, :]
                                )
                else:
                    for ao in range(AO):
                        vproj_tile(ao)
                    for ao in range(AO):
                        qproj_tile(ao)

                proj.release()
                psp.release()
                if tp:
                    dram.release()
                ap = tc.alloc_tile_pool(name="ap", bufs=1)
                psa = tc.alloc_tile_pool(name="psa", bufs=1, space="PSUM")

                # ---- attention: 3-stage software pipeline over q-tiles ----
                # A(i): score halves [P,1024] (PSUM tag sc bufs=3) + per-half
                #       DVE max reduces + combine -> nm(i)
                # B(i): ACT exp halves (+row-sum accum) + per-half DMA-XBAR
                #       transposes + DVE recip
                # C(i): ctx matmuls (PSUM cx bufs=1) + DVE 1/sum scale + out
                # Emission A(0) A(1) B(0) [A(i) B(i-1) C(i-2)]... keeps each
                # in-order engine queue free of cross-stage back-waits.
                def sc_stage(qi):
                    halves = []
                    maxes = []
                    for hf in range(2):
                        sc = psa.tile(
                            [P, 1024], F32, name=f"sc_{qi}_{hf}", tag="sc", bufs=3
                        )
                        for ach in range(AO):
                            for c2 in range(2):
                                nc.tensor.matmul(
                                    sc[:, c2 * 512 : (c2 + 1) * 512],
                                    qT[:, ach, qi * P : (qi + 1) * P],
                                    vT[:, ach, hf * 1024 + c2 * 512 : hf * 1024 + (c2 + 1) * 512],
                                    start=(ach == 0),
                                    stop=(ach == AO - 1),
                                )
                        m = ap.tile([P, 1], F32, name=f"m_{qi}_{hf}", tag=f"m{hf}", bufs=2)
                        nc.vector.tensor_reduce(m[:], sc[:], AxX, MaxOp)
                        halves.append(sc)
                        maxes.append(m)
                    nm = ap.tile([P, 1], F32, name=f"nm_{qi}", tag="nm", bufs=2)
                    nc.vector.tensor_scalar_max(nm[:], maxes[0][:], maxes[1][:])
                    nc.vector.tensor_scalar_mul(nm[:], nm[:], -1.0)
                    return halves, nm

                def exp_stage(qi, halves, nm):
                    attn = ap.tile([P, S], F16, name=f"at_{qi}", tag="attn", bufs=2)
                    attnT = ap.tile([P, SO, P], F16, name=f"aT_{qi}", tag="aT", bufs=2)
                    s0 = ap.tile([P, 1], F32, name=f"s0_{qi}", tag="s0", bufs=2)
                    s1 = ap.tile([P, 1], F32, name=f"s1_{qi}", tag="s1", bufs=2)
                    for hf, acc in ((0, s0), (1, s1)):
                        nc.scalar.activation(
                            attn[:, hf * 1024 : (hf + 1) * 1024], halves[hf][:],
                            Exp, bias=nm[:], accum_out=acc[:],
                        )
                        nc.sync.dma_start_transpose(
                            attnT[:, hf * 8 : (hf + 1) * 8, :],
                            attn[:, hf * 1024 : (hf + 1) * 1024],
                        )
                    recip = ap.tile([P, 1], F32, name=f"rc_{qi}", tag="rc", bufs=2)
                    nc.vector.tensor_add(recip[:], s0[:], s1[:])
                    nc.vector.reciprocal(recip[:], recip[:])
                    return attnT, recip

                def ctx_stage(qi, attnT, recip):
                    cx = psa.tile([P, A], F32, name=f"cx_{qi}", tag="cx", bufs=1)
                    for kb in range(SO):
                        for c2 in range(2):
                            nc.tensor.matmul(
                                cx[:, c2 * 512 : (c2 + 1) * 512],
                                attnT[:, kb, :],
                                v_sb[:, kb, c2 * 512 : (c2 + 1) * 512],
                                start=(kb == 0),
                                stop=(kb == SO - 1),
                            )
                    outt = ap.tile([P, A], F16, name=f"ot_{qi}", tag="ot", bufs=2)
                    nc.vector.tensor_scalar_mul(outt[:], cx[:], recip[:])
                    nc.gpsimd.dma_start(out_t[qi], outt[:])

                for _ra in range(ra):
                    Aq = {0: sc_stage(0), 1: sc_stage(1)}
                    Bq = {0: exp_stage(0, *Aq.pop(0))}
                    for qi in range(2, QO):
                        Aq[qi] = sc_stage(qi)
                        Bq[qi - 1] = exp_stage(qi - 1, *Aq.pop(qi - 1))
                        ctx_stage(qi - 2, *Bq.pop(qi - 2))
                    Bq[QO - 1] = exp_stage(QO - 1, *Aq.pop(QO - 1))
                    ctx_stage(QO - 2, *Bq.pop(QO - 2))
                    ctx_stage(QO - 1, *Bq.pop(QO - 1))

                ap.release()
                psa.release()

    nc.compile()
    return nc


def make_in_maps(inputs, tp=None):
    """Shard FULL inputs into per-core input maps (host-side, untimed)."""
    if tp is None:
        tp = TP
    query = np.asarray(inputs["query"], dtype=np.float32)
    value = np.asarray(inputs["value"], dtype=np.float32)
    Wq = np.asarray(inputs["Wq"], dtype=np.float32)
    Wv = np.asarray(inputs["Wv"], dtype=np.float32)
    bqv = np.asarray(inputs["bq"], dtype=np.float32)
    bvv = np.asarray(inputs["bv"], dtype=np.float32)

    q16 = query.astype(np.float16)
    v16 = value.astype(np.float16)
    # weight pre-tiling (pure layout): [H, A] -> [128, H//128, A]
    wq_t = np.ascontiguousarray(
        Wq.reshape(KO, P, A).transpose(1, 0, 2).astype(np.float16)
    )
    wv_t = np.ascontiguousarray(
        Wv.reshape(KO, P, A).transpose(1, 0, 2).astype(np.float16)
    )
    bq_t = np.ascontiguousarray(bqv.reshape(AO, P).T)
    bv_t = np.ascontiguousarray(bvv.reshape(AO, P).T)

    in_maps = []
    for c in range(N_CORES):
        b, h = c // 2, c % 2
        if tp:
            wv_c = np.ascontiguousarray(wv_t[:, :, h * (A // 2) : (h + 1) * (A // 2)])
            bv_c = np.ascontiguousarray(bv_t[:, h * (AO // 2) : (h + 1) * (AO // 2)])
        else:
            wv_c, bv_c = wv_t, bv_t
        in_maps.append(
            {
                "xq": np.ascontiguousarray(q16[b, h * SQ : (h + 1) * SQ, :]),
                "xv": np.ascontiguousarray(v16[b]),
                "wq": wq_t,
                "wv": wv_c,
                "bq": bq_t,
                "bv": bv_c,
            }
        )
    return in_maps


_NC_CACHE = {}


def _get_nc():
    if "nc" not in _NC_CACHE:
        _NC_CACHE["nc"] = build()
    return _NC_CACHE["nc"]


def kernel(**inputs):
    nc = _get_nc()
    in_maps = make_in_maps(inputs)
    res = run_bass_kernel_spmd(nc, in_maps, core_ids=list(range(N_CORES)))
    out = np.empty((B, S, A), np.float32)
    for c in range(N_CORES):
        b, h = c // 2, c % 2
        out[b, h * SQ : (h + 1) * SQ, :] = res.results[c]["out"]  # f16 -> f32
    return out

